# revision 48
# baseline (speedup 1.0000x reference)
"""Per-pixel adaptive (kernel-prediction) 5x5 conv on 8 trn2 cores.

out[b,c,y,x] = sum_{i,j} x_pad[b,c,y+i,x+j] * kernel[b,(c*5+i)*5+j,y,x]
with edge (replication) padding p=2.

Sharding: 8 cores = B(4) x C-halves(2).  The op is depthwise (output
channel c reads only input channel c), so slicing C needs no halo.

Device layout: 128 SBUF partitions = 16 channels x 8 row-groups of 32
rows.  Rows are processed in 4 quarter-passes of 8 rows (2048 output
elems per partition per quarter = 4 PSUM banks, so two quarters
ping-pong in PSUM).

The kernel tensor dominates HBM traffic (52.4 MB f16 per core).  It is
pre-swizzled ON THE HOST into [20 chunks][128 partitions][5 taps x 2048]
so each chunk DMA reads one 20 KB contiguous block per partition
(descriptor size 20 KB instead of 8 KB -> ~26.8 vs ~15 GB/s per SDMA
engine; 20 KB was measured faster than both 8 KB and 40 KB).  Chunks
alternate between the two HWDGE queues (sync / scalar); x and the
output stores ride the SWDGE (gpsimd) queue so they never delay a
chunk load.

Per tap: DVE computes the f16 product into a tmp tile (the serialized
~122 us of DVE time is the compute-side floor; GpSimd tensor ops were
tried and are both ~4x slower and degrade DVE throughput via SBUF
contention).  The otherwise idle TensorE accumulates tmp into PSUM via
identity matmuls (PSUM accumulate-on-write does the adds).  ScalarE
drains PSUM -> SBUF; gpsimd SWDGE stores to DRAM.
"""

import numpy as np

B, C, H, W, K = 4, 32, 256, 256, 5
P = (K - 1) // 2   # 2
CP = 16            # channels per core
YG = 8             # row groups per partition-set
RG = H // YG       # 32 rows per group
WP = W + 2 * P     # 260
SROWS = RG + 2 * P  # 36 rows per x stripe
SLEN = SROWS * WP   # 9360 elems per partition x-stripe
NQ = 4             # quarter-passes per row group
QR = RG // NQ      # 8 rows per quarter
QFREE = QR * W     # 2048 free elems per quarter
NTG = 5            # tap groups (of 5 taps each)
CHUNK = NTG * QFREE  # 10240 elems per partition per chunk DMA
NCHUNK = NQ * NTG    # 20 chunk DMAs

_cache = {}


def _build_nc():
    import concourse.bass as bass
    import concourse.tile as tile
    from concourse import bacc, mybir

    f32 = mybir.dt.float32
    f16 = mybir.dt.float16
    nc = bacc.Bacc("TRN2", target_bir_lowering=False, debug=False, num_devices=8)

    xsw_t = nc.dram_tensor("xsw", [128, SLEN], f16, kind="ExternalInput")
    ksw_t = nc.dram_tensor("ksw", [NCHUNK, 128, CHUNK], f16, kind="ExternalInput")
    ident_t = nc.dram_tensor("ident", [128, 128], f16, kind="ExternalInput")
    out_t = nc.dram_tensor("out", [NQ, 128, QFREE], f16, kind="ExternalOutput")

    with tile.TileContext(nc) as tc:
        with (
            tc.tile_pool(name="xp", bufs=1) as xpool,
            tc.tile_pool(name="idp", bufs=1) as ipool,
            tc.tile_pool(name="kp", bufs=6) as kpool,
            tc.tile_pool(name="tp", bufs=6) as tpool,
            tc.tile_pool(name="op", bufs=4) as opool,
            tc.tile_pool(name="pp", bufs=2, space="PSUM") as ppool,
        ):
            # Startup: x halves + chunk0 halves head BOTH HWDGE rings, so
            # the first-needed data is never starved by per-packet
            # round-robin against later chunks.  Chunks 1+ alternate rings
            # whole (20KB descriptors = best packet rate).
            xtile = xpool.tile([128, SLEN], f16)
            XH = 14 * WP  # rows 0-13: covers all q=0 reads (rows <= 11)
            nc.sync.dma_start(
                out=xtile[:, :XH],
                in_=bass.AP(xsw_t, 0, [[SLEN, 128], [1, XH]]),
                single_packet=True,
            )
            ident = ipool.tile([128, 128], f16)
            # x tail (rows 14+, first read ~49us in by chunk 5) rides the
            # SWDGE queue: it lands ~27us in and, unlike a ring placement,
            # cannot be hoisted ahead of a kernel-chunk DMA by the
            # scheduler (that starved chunk3 by ~14us).
            nc.gpsimd.dma_start(
                out=xtile[:, XH:],
                in_=bass.AP(xsw_t, XH, [[SLEN, 128], [1, SLEN - XH]]),
            )

            x3 = xtile[:].rearrange("p (r w) -> p r w", w=WP)

            ASPLIT = 3 * QFREE  # chunk0: taps 0-2 -> sync, taps 3-4 -> scalar
            pending_drains = []

            def emit_drain(q, ptile):
                # PSUM -> SBUF -> DRAM.  The last quarter drains in 4
                # pipelined pieces onto the (by then empty) HWDGE rings to
                # shrink the end-of-kernel tail; mid-stream quarters use one
                # ScalarE copy + a SWDGE store so the rings stay clear.
                if q == NQ - 1:
                    NS = 4
                    SP = QFREE // NS
                    for si in range(NS):
                        obuf = opool.tile([128, SP], f16, tag="ob")
                        # drain on ScalarE and the (now idle) DVE in parallel
                        if si % 2 == 0:
                            nc.scalar.copy(
                                obuf[:], ptile[:, si * SP : (si + 1) * SP]
                            )
                        else:
                            nc.vector.tensor_copy(
                                obuf[:], ptile[:, si * SP : (si + 1) * SP]
                            )
                        dst = bass.AP(
                            out_t,
                            q * 128 * QFREE + si * SP,
                            [[QFREE, 128], [1, SP]],
                        )
                        oeng = nc.sync if si % 2 == 0 else nc.scalar
                        oeng.dma_start(out=dst, in_=obuf[:])
                else:
                    obuf = opool.tile([128, QFREE], f16, tag="ob")
                    nc.scalar.copy(obuf[:], ptile[:])
                    dst = bass.AP(
                        out_t, q * 128 * QFREE, [[QFREE, 128], [1, QFREE]]
                    )
                    nc.gpsimd.dma_start(out=dst, in_=obuf[:])

            for chunk in range(NCHUNK):
                q, tg = divmod(chunk, NTG)
                ktile = kpool.tile([128, CHUNK], f16, tag="kt")
                base = chunk * 128 * CHUNK
                if chunk < 2:
                    # chunks 0-1 split across both rings: the pipeline
                    # primes symmetrically, so chunk1 lands well before its
                    # first DVE op (whole-chunk1-on-scalar arrived ~just as
                    # op5 needed it, a recurring 4-12us stall).
                    nc.sync.dma_start(
                        out=ktile[:, :ASPLIT],
                        in_=bass.AP(ksw_t, base, [[CHUNK, 128], [1, ASPLIT]]),
                        single_packet=True,
                    )
                    nc.scalar.dma_start(
                        out=ktile[:, ASPLIT:],
                        in_=bass.AP(
                            ksw_t,
                            base + ASPLIT,
                            [[CHUNK, 128], [1, CHUNK - ASPLIT]],
                        ),
                        single_packet=True,
                    )
                    if chunk == 0:
                        nc.scalar.dma_start(out=ident[:], in_=ident_t[:, :])
                else:
                    keng = nc.scalar if chunk % 2 == 1 else nc.sync
                    keng.dma_start(
                        out=ktile[:],
                        in_=bass.AP(ksw_t, base, [[CHUNK, 128], [1, CHUNK]]),
                        single_packet=True,
                    )

                # deferred drains: emitted AFTER this chunk's DMA issue so a
                # drain's stop-matmul wait can never block a chunk-DMA issue
                # (inline drains systematically lag the scalar ring).
                while pending_drains and pending_drains[0][0] + 3 <= chunk:
                    emit_drain(*pending_drains.pop(0)[1])

                if tg == 0:
                    ptile = ppool.tile([128, QFREE], f32, tag="ps")

                for t in range(NTG):
                    ij = tg * NTG + t
                    i, j = divmod(ij, K)
                    xv = x3[:, q * QR + i : q * QR + i + QR, j : j + W]
                    k3 = ktile[:, t * QFREE : (t + 1) * QFREE].rearrange(
                        "p (r w) -> p r w", w=W
                    )
                    tmp = tpool.tile([128, QFREE], f16, tag="tmp")
                    t3 = tmp[:].rearrange("p (r w) -> p r w", w=W)
                    nc.vector.tensor_mul(t3, xv, k3)
                    for bk in range(QFREE // 512):
                        nc.tensor.matmul(
                            out=ptile[:, bk * 512 : (bk + 1) * 512],
                            lhsT=ident[:],
                            rhs=tmp[:, bk * 512 : (bk + 1) * 512],
                            start=(ij == 0),
                            stop=(ij == K * K - 1),
                        )

                if tg == NTG - 1:
                    pending_drains.append((chunk, (q, ptile)))

            for _, args in pending_drains:
                emit_drain(*args)

    nc.compile()
    return nc


def _get_nc():
    if "nc" not in _cache:
        _cache["nc"] = _build_nc()
    return _cache["nc"]


_IDENT = np.eye(128, dtype=np.float16)

# row index grid for the overlapping padded x stripes: (YG, SROWS)
_ROWS = (np.arange(YG)[:, None] * RG + np.arange(SROWS)[None, :])


def prepare_in_maps(x, kern):
    """Host-side shard + swizzle.  x, kern: full f32 arrays."""
    x = np.asarray(x, dtype=np.float32).astype(np.float16)
    kern = np.asarray(kern, dtype=np.float32).astype(np.float16)
    xpad = np.pad(x, ((0, 0), (0, 0), (P, P), (P, P)), mode="edge")

    in_maps = []
    for core in range(8):
        b, half = divmod(core, 2)
        c0 = half * CP
        # x stripes: (CP, YG, SROWS, WP) -> (128, SLEN)
        xs = xpad[b, c0 : c0 + CP][:, _ROWS, :].reshape(128, SLEN)
        # kernel swizzle: (CP*K*K, H, W) ->
        # (c, tg, t, g, q, r, w) -> (q, tg, c, g, t, r, w) -> (20, 128, CHUNK)
        kc = kern[b, c0 * K * K : (c0 + CP) * K * K].reshape(
            CP, NTG, K, YG, NQ, QR, W
        )
        ks = np.ascontiguousarray(kc.transpose(4, 1, 0, 3, 2, 5, 6)).reshape(
            NCHUNK, 128, CHUNK
        )
        in_maps.append({"xsw": np.ascontiguousarray(xs), "ksw": ks, "ident": _IDENT})
    return in_maps


def kernel(x, kernel, kernel_size):
    from concourse.bass_utils import run_bass_kernel_spmd

    in_maps = prepare_in_maps(x, kernel)
    nc = _get_nc()
    res = run_bass_kernel_spmd(nc, in_maps, list(range(8)))

    out = np.empty((B, C, H, W), dtype=np.float32)
    for core in range(8):
        b, half = divmod(core, 2)
        c0 = half * CP
        # out_sw: (NQ, 128, QFREE) -> (q, c, g, r, w) -> (c, g, q, r, w)
        osw = res.results[core]["out"].reshape(NQ, CP, YG, QR, W)
        out[b, c0 : c0 + CP] = (
            osw.transpose(1, 2, 0, 3, 4).reshape(CP, H, W).astype(np.float32)
        )
    return out
